# revision 1
# baseline (speedup 1.0000x reference)
"""Trainium2 Bass kernel for nn_BoxMinusMatNLLLoss.

Reference (per element n of N = B*T = 1024*1024):
    qd   = [x, y, th, th],  x = q0-qh0, y = q1-qh1,
           th = vpi(q2 - atan2(qh2, qh3 + eps*(qh3==0)))
    l_traj = 0.5 * qd^T inv(C) qd          (C symmetric SPD 4x4)
    l_cov  = 0.5 * log(||C||_F) = 0.25 * log(sum(C^2))
    out = mean(l_traj) + mean(l_cov)       (scalar f32)

Kernel design (pure data parallel, 8 cores; per core 131072 elements as
[128 partitions, E=1024]):

  * atan2 by range reduction: r = min(|y|,|x|)/max(|y|,|x|) in [0,1]
    (the ACT Arctan table only covers [-pi/2, pi/2]), and since the
    downstream wrap works mod 2pi:
        atan2(y,x) == sign(x*y)*phi + pi*(x<0)   (mod 2pi),
        phi = |swap*pi/2 - arctan(r)|.
  * wrap: th = delta - 2pi*round(delta/(2pi)) via f32->int32 copy
    (hardware rounds RTNE; the CoreSim truncates, so sims build the
    "trunc" variant with a +8.5 offset).
  * qd = [x,y,t,t] Schur-reduces the 4x4 solve to a 3x3 LDL^T:
        QF = u^T S^-1 u, u = (x,y,t), S = A - b b^T/dt,
        b_i = C[i,3]-C[i,2], dt = C22+C33-2*C23.
  * All reciprocals (5/element, all positive arguments) run on the ACT
    engine as exp(-ln(x)) — Ln and Exp share one table set, and the DVE
    InstReciprocal measures ~20x slower than an elementwise op.
  * l_cov: sum(C^2) = sum(diag^2) + 2 sum(offdiag^2) via ACT Square
    (scale=sqrt(2) folds the 2), segmented-reduced on GPSIMD (tree adds)
    and DVE (tensor_reduce), then ACT Ln with accum_out.
  * Partial sums leave the chip as a [128, nchunk-ish] tile per core via
    accum_out; the host sums 8 small tiles and applies 0.5/N, 0.25/N.
"""

import math

import numpy as np

import concourse.bass as bass
import concourse.tile as tile
from concourse import mybir
from concourse.bass_utils import run_bass_kernel_spmd

F32 = mybir.dt.float32
I32 = mybir.dt.int32
BF16 = mybir.dt.bfloat16
PI = math.pi
NCORES = 8
P = 128

# f32 -> int32 copy semantics: hardware = "rtne" (probe-verified);
# CoreSim = "trunc".
INT_CONV = "rtne"
# "act" = exp(-ln x) pairs on ScalarE; "dve" = InstReciprocal (slow).
RECIP_MODE = "act"


def _sub_ap(t, f_count, inner_off, inner_step, inner_cnt):
    """[P, f_count, inner_cnt] view of a [P, f_count, K] tile with custom
    inner offset/step (e.g. matrix diagonals)."""
    a = t[:, :, :]
    return bass.AP(
        tensor=a.tensor,
        offset=a.offset + inner_off * a.ap[2][0],
        ap=[a.ap[0], [a.ap[1][0], f_count], [a.ap[2][0] * inner_step, inner_cnt]],
    )


def _bc(ap2d, n):
    """Broadcast a [P, F] AP to [P, F, n] with step 0 on the inner dim."""
    return bass.AP(
        tensor=ap2d.tensor,
        offset=ap2d.offset,
        ap=[ap2d.ap[0], ap2d.ap[1], [0, n]],
    )


def split_multi_waits(nc):
    """The walrus build in this container encodes only one sync wait per
    instruction; Tile's tail drain carries several.  Split extras into
    single-wait NOPs placed just before."""
    for fn in nc.m.functions:
        for bb in fn.blocks:
            new_insts = []
            for ins in bb.instructions:
                si = ins.sync_info
                if si is not None and si.on_wait and len(si.on_wait) > 1:
                    waits = list(si.on_wait)
                    for w in waits[:-1]:
                        nop = mybir.InstNoOp(
                            name=nc.get_next_instruction_name(), ins=[], outs=[]
                        )
                        nop.engine = ins.engine
                        nop.sync_info = mybir.SyncInfo(on_wait=[w], on_update=[])
                        new_insts.append(nop)
                    si.on_wait = [waits[-1]]
                new_insts.append(ins)
            bb.instructions = new_insts


def build_nc(E=1024, F=512, F3=256, gps_tree_subchunks=(0, 1, 2),
             split_waits=True, bufs=None, repeat=1, opts=None):
    """Build the per-core Bass program (see module docstring)."""
    assert E % F == 0 and E % F3 == 0
    nch = E // F
    nch3 = E // F3
    nqf_per = 3 if (opts or {}).get("p2_bf16") else 1
    nqf = nch * nqf_per
    ncols = nqf + nch3  # qf partials + ln partials
    AT = mybir.ActivationFunctionType
    OP = mybir.AluOpType
    AX = mybir.AxisListType

    nc = bass.Bass()
    q = nc.declare_dram_parameter("q", [P, E * 4], F32, isOutput=False)
    qh = nc.declare_dram_parameter("q_hat", [P, E * 4], F32, isOutput=False)
    cov = nc.declare_dram_parameter("cov", [P, E * 16], F32, isOutput=False)
    out = nc.declare_dram_parameter("out", [P, ncols], F32, isOutput=True)

    qv = q.rearrange("p (e c) -> p e c", c=4)
    qhv = qh.rearrange("p (e c) -> p e c", c=4)
    covv = cov.rearrange("p (e c) -> p e c", c=16)

    B = dict(inp=2, inq=1, sc1=6, rc=4, p3s=4, sc2=4, sc3=4, long=2,
             p3=2, p31=1, acc=1, psum=2)
    if bufs:
        B.update(bufs)
    O = dict(abs_dve=False, sgm_dve=False, pr01_gps=False,
             skip_p1=False, skip_p2=False, skip_p3=False, p2_bf16=False,
             wsq_dve=False, p3sq_dve=False, phi_dve=False, p3early=False,
             p3v2=False, p3mid=False)
    if opts:
        O.update(opts)
    with tile.TileContext(nc) as tc:
        with (
            tc.tile_pool(name="inp", bufs=B["inp"]) as inp,
            tc.tile_pool(name="inq", bufs=B["inq"]) as inq,
            tc.tile_pool(name="sc1", bufs=B["sc1"]) as sc1,
            tc.tile_pool(name="rc", bufs=B["rc"]) as rcp,
            tc.tile_pool(name="p3s", bufs=B["p3s"]) as p3sp,
            tc.tile_pool(name="sc2", bufs=B["sc2"]) as sc2,
            tc.tile_pool(name="sc3", bufs=B["sc3"]) as sc3,
            tc.tile_pool(name="long", bufs=B["long"]) as lng,
            tc.tile_pool(name="p3", bufs=B["p3"]) as p3p,
            tc.tile_pool(name="p31", bufs=B["p31"]) as p31,
            tc.tile_pool(name="acc", bufs=B["acc"]) as accp,
            tc.tile_pool(name="psum", bufs=B["psum"], space="PSUM") as psp,
        ):
            def _body():
                outacc = accp.tile([P, ncols], F32)
                nc.vector.memset(outacc, 0.0)

                def recip(out_ap, in_ap, shape):
                    if RECIP_MODE == "dve":
                        return nc.vector.reciprocal(out=out_ap, in_=in_ap)
                    lnt = rcp.tile(shape, F32, tag="rc")
                    ln_i = nc.scalar.activation(out=lnt, in_=in_ap, func=AT.Ln)
                    nc.scalar.activation(out=out_ap, in_=lnt, func=AT.Exp,
                                         scale=-1.0)
                    return ln_i

                q_ts, qh_ts, cov_ts = [], [], []
                for c in range(nch):
                    lo = c * F
                    q_t = inq.tile([P, F, 4], F32, tag="q")
                    qh_t = inq.tile([P, F, 4], F32, tag="qh")
                    nc.sync.dma_start(out=q_t, in_=qv[:, lo : lo + F, :])
                    nc.sync.dma_start(out=qh_t, in_=qhv[:, lo : lo + F, :])
                    cov_t = inp.tile([P, F, 16], F32, tag="cov")
                    nc.sync.dma_start(out=cov_t, in_=covv[:, lo : lo + F, :])
                    q_ts.append(q_t)
                    qh_ts.append(qh_t)
                    cov_ts.append(cov_t)

                # ------- P1a: |.|, min/max, swap flag, 1/mx, r  (ln/exp set)
                p1 = []
                rmx_lns, atan_is, p2_lns = [], [], []
                for c in range(nch):
                    q_t, qh_t = q_ts[c], qh_ts[c]
                    if O["p2_bf16"]:
                        wpack = lng.tile([P, F], BF16, tag="wpack")  # x
                        xslot = wpack
                    else:
                        wpack = lng.tile([P, F, 3], F32, tag="wpack")
                        xslot = wpack[:, :, 0]
                    ytile = lng.tile([P, F], BF16 if O["p2_bf16"] else F32,
                                     tag="ytile")
                    nc.gpsimd.tensor_tensor(
                        out=xslot, in0=q_t[:, :, 0], in1=qh_t[:, :, 0],
                        op=OP.subtract)
                    nc.gpsimd.tensor_tensor(
                        out=ytile, in0=q_t[:, :, 1], in1=qh_t[:, :, 1],
                        op=OP.subtract)
                    ay = sc1.tile([P, F], F32, tag="sc1")
                    nc.scalar.activation(out=ay, in_=qh_t[:, :, 2], func=AT.Abs)
                    ax = sc1.tile([P, F], F32, tag="sc1")
                    nc.scalar.activation(out=ax, in_=qh_t[:, :, 3], func=AT.Abs)
                    mx = sc1.tile([P, F], F32, tag="sc1")
                    nc.vector.tensor_tensor(out=mx, in0=ax, in1=ay, op=OP.max)
                    mn = sc1.tile([P, F], F32, tag="sc1")
                    nc.vector.tensor_tensor(out=mn, in0=ax, in1=ay, op=OP.min)
                    sw = lng.tile([P, F], F32, tag="sw")
                    nc.vector.tensor_tensor(out=sw, in0=ay, in1=ax, op=OP.is_gt)
                    rmx = sc1.tile([P, F], F32, tag="sc1")
                    rmx_lns.append(recip(rmx, mx, [P, F]))
                    rr = lng.tile([P, F], F32, tag="rr")
                    nc.vector.tensor_tensor(out=rr, in0=mn, in1=rmx, op=OP.mult)
                    xy = lng.tile([P, F], F32, tag="xy")
                    nc.vector.tensor_tensor(
                        out=xy, in0=qh_t[:, :, 2], in1=qh_t[:, :, 3], op=OP.mult)
                    neg = sc1.tile([P, F], F32, tag="sc1")
                    nc.vector.tensor_scalar(
                        out=neg, in0=qh_t[:, :, 3], scalar1=0.0, scalar2=None,
                        op0=OP.is_lt)
                    base = lng.tile([P, F], F32, tag="base")
                    nc.vector.scalar_tensor_tensor(
                        out=base, in0=neg, scalar=-PI, in1=q_t[:, :, 2],
                        op0=OP.mult, op1=OP.add)
                    p1.append((wpack, ytile, sw, rr, xy, base))

                def emit_p3_v2(sel=None):
                    sub_per_chunk = F // F3
                    for s in range(nch3 if not O["skip_p3"] else 0):
                        c = s // sub_per_chunk
                        if sel is not None and c != sel:
                            continue
                        off = (s % sub_per_chunk) * F3
                        cov_t = cov_ts[c]
                        # component-major: sq10v[:, k*F3:(k+1)*F3] = comp k
                        sq10v = p3p.tile([P, 10 * F3], F32, tag="sq10")
                        cs = cov_t[:, :, :]

                        def cmaj(o, st, n):
                            # in: comps o, o+st, ... (n comps) over F3 elements
                            return bass.AP(
                                tensor=cs.tensor,
                                offset=cs.offset + (off * 16 + o),
                                ap=[cs.ap[0], [16, F3], [st, n]],
                            )

                        def omaj(k0, n):
                            a = sq10v[:, :]
                            return bass.AP(
                                tensor=a.tensor,
                                offset=a.offset + k0 * F3,
                                ap=[a.ap[0], [1, F3], [F3, n]],
                            )

                        r2s = math.sqrt(2.0)
                        nc.scalar.activation(
                            out=omaj(0, 4), in_=cmaj(0, 5, 4), func=AT.Square)
                        nc.scalar.activation(
                            out=omaj(4, 3), in_=cmaj(1, 1, 3), func=AT.Square,
                            scale=r2s)
                        nc.scalar.activation(
                            out=omaj(7, 2), in_=cmaj(6, 1, 2), func=AT.Square,
                            scale=r2s)
                        nc.scalar.activation(
                            out=omaj(9, 1), in_=cmaj(11, 1, 1), func=AT.Square,
                            scale=r2s)
                        t5 = p31.tile([P, 5 * F3], F32, tag="t5")
                        nc.gpsimd.tensor_tensor(
                            out=t5, in0=sq10v[:, 0 : 5 * F3],
                            in1=sq10v[:, 5 * F3 : 10 * F3], op=OP.add)
                        t2t = p31.tile([P, 2 * F3], F32, tag="t2t")
                        nc.gpsimd.tensor_tensor(
                            out=t2t, in0=t5[:, 0 : 2 * F3],
                            in1=t5[:, 2 * F3 : 4 * F3], op=OP.add)
                        sa = p3sp.tile([P, F3], F32, tag="p3s")
                        nc.gpsimd.tensor_tensor(
                            out=sa, in0=t2t[:, 0:F3], in1=t2t[:, F3 : 2 * F3],
                            op=OP.add)
                        ssq = p3sp.tile([P, F3], F32, tag="p3s")
                        nc.gpsimd.tensor_tensor(
                            out=ssq, in0=sa, in1=t5[:, 4 * F3 : 5 * F3],
                            op=OP.add)
                        lnsc = p3sp.tile([P, F3], F32, tag="p3s")
                        p2_lns.append(nc.scalar.activation(
                            out=lnsc, in_=ssq, func=AT.Ln,
                            accum_out=outacc[:, nqf + s : nqf + s + 1]))

                def emit_p3():
                    if O["p3v2"]:
                        return emit_p3_v2()
                    # ------- P3: l_cov  (ln/exp set)
                    sub_per_chunk = F // F3
                    for s in range(nch3 if not O["skip_p3"] else 0):
                        c = s // sub_per_chunk
                        off = (s % sub_per_chunk) * F3
                        cov_t = cov_ts[c]
                        sq10 = p3p.tile([P, F3, 10], F32, tag="sq10")
                        cs = cov_t[:, :, :]

                        def cslice(o, st, n):
                            return bass.AP(
                                tensor=cs.tensor,
                                offset=cs.offset + (off * 16 + o),
                                ap=[cs.ap[0], [16, F3], [st, n]],
                            )

                        r2s = math.sqrt(2.0)
                        nc.scalar.activation(
                            out=sq10[:, :, 0:4], in_=cslice(0, 5, 4), func=AT.Square)
                        nc.scalar.activation(
                            out=sq10[:, :, 7:9], in_=cslice(6, 1, 2), func=AT.Square,
                            scale=r2s)
                        if O["p3sq_dve"]:
                            # 2*c^2 via one STT: (c*2) mult c
                            nc.vector.scalar_tensor_tensor(
                                out=sq10[:, :, 4:7], in0=cslice(1, 1, 3), scalar=2.0,
                                in1=cslice(1, 1, 3), op0=OP.mult, op1=OP.mult)
                            nc.vector.scalar_tensor_tensor(
                                out=sq10[:, :, 9:10], in0=cslice(11, 1, 1), scalar=2.0,
                                in1=cslice(11, 1, 1), op0=OP.mult, op1=OP.mult)
                        else:
                            nc.scalar.activation(
                                out=sq10[:, :, 4:7], in_=cslice(1, 1, 3),
                                func=AT.Square, scale=r2s)
                            nc.scalar.activation(
                                out=sq10[:, :, 9:10], in_=cslice(11, 1, 1),
                                func=AT.Square, scale=r2s)
                        ssq = p3sp.tile([P, F3], F32, tag="p3s")
                        if (s % nch3) in gps_tree_subchunks:
                            t5 = p31.tile([P, F3, 5], F32, tag="t5")
                            nc.gpsimd.tensor_tensor(
                                out=t5, in0=sq10[:, :, 0:5], in1=sq10[:, :, 5:10],
                                op=OP.add)
                            t2t = p31.tile([P, F3, 2], F32, tag="t2t")
                            nc.gpsimd.tensor_tensor(
                                out=t2t, in0=t5[:, :, 0:2], in1=t5[:, :, 2:4], op=OP.add)
                            sa = p3sp.tile([P, F3], F32, tag="p3s")
                            nc.gpsimd.tensor_tensor(
                                out=sa, in0=t2t[:, :, 0], in1=t2t[:, :, 1], op=OP.add)
                            nc.gpsimd.tensor_tensor(
                                out=ssq, in0=sa, in1=t5[:, :, 4], op=OP.add)
                        else:
                            nc.vector.tensor_reduce(
                                out=ssq, in_=sq10, axis=AX.X, op=OP.add)
                        lnsc = p3sp.tile([P, F3], F32, tag="p3s")
                        p2_lns.append(nc.scalar.activation(
                            out=lnsc, in_=ssq, func=AT.Ln,
                            accum_out=outacc[:, nqf + s : nqf + s + 1]))


                if O["p3early"]:
                    emit_p3()

                # ------- P1b: arctan (trig set), quadrant fold, wrap
                ths = []
                for c in range(nch):
                    wpack, ytile, sw, rr, xy, base = p1[c]
                    if O["skip_p1"]:
                        ths.append(base)
                        continue
                    at = sc1.tile([P, F], F32, tag="sc1")
                    atan_is.append(
                        nc.scalar.activation(out=at, in_=rr, func=AT.Arctan))
                    t1 = sc1.tile([P, F], F32, tag="sc1")
                    nc.vector.scalar_tensor_tensor(
                        out=t1, in0=sw, scalar=PI / 2, in1=at,
                        op0=OP.mult, op1=OP.subtract)
                    phi = sc1.tile([P, F], F32, tag="sc1")
                    if O["phi_dve"]:
                        nc.vector.tensor_scalar(
                            out=phi.bitcast(I32), in0=t1.bitcast(I32),
                            scalar1=0x7FFFFFFF, scalar2=None, op0=OP.bitwise_and)
                    else:
                        nc.scalar.activation(out=phi, in_=t1, func=AT.Abs)
                    sgm = sc1.tile([P, F], F32, tag="sc1")
                    if O["sgm_dve"]:
                        nc.vector.tensor_scalar(
                            out=sgm, in0=xy, scalar1=0.0, scalar2=2.0,
                            op0=OP.is_ge, op1=OP.mult)
                        nc.vector.tensor_scalar(
                            out=sgm, in0=sgm, scalar1=1.0, scalar2=None,
                            op0=OP.subtract)
                    else:
                        nc.scalar.activation(out=sgm, in_=xy, func=AT.Sign)
                    mp = sc1.tile([P, F], F32, tag="sc1")
                    nc.vector.tensor_tensor(out=mp, in0=sgm, in1=phi, op=OP.mult)
                    delta = sc1.tile([P, F], F32, tag="sc1")
                    nc.vector.tensor_tensor(
                        out=delta, in0=base, in1=mp, op=OP.subtract)
                    # th = delta - 2pi*round(delta/2pi); trunc variant offsets
                    # by +8.5 (z>0 so trunc==floor) and folds -16pi into w3.
                    zbias = 0.0 if INT_CONV == "rtne" else 8.5
                    z = sc1.tile([P, F], F32, tag="sc1")
                    nc.vector.tensor_scalar(
                        out=z, in0=delta, scalar1=1.0 / (2 * PI), scalar2=zbias,
                        op0=OP.mult, op1=OP.add)
                    fi = sc1.tile([P, F], I32, tag="sc1")
                    nc.vector.tensor_copy(fi, z)
                    ff = sc1.tile([P, F], F32, tag="sc1")
                    nc.vector.tensor_copy(ff, fi)
                    th = lng.tile([P, F], BF16 if O["p2_bf16"] else F32,
                                  tag="th")
                    nc.vector.scalar_tensor_tensor(
                        out=th, in0=ff, scalar=-2 * PI, in1=delta,
                        op0=OP.mult, op1=OP.add)
                    ths.append(th)

                # ------- P2: Schur + LDL + forward + QF  (ln/exp set)
                for c in range(nch):
                    if O["skip_p2"]:
                        break
                    cov_t = cov_ts[c]
                    wpack, ytile = p1[c][0], p1[c][1]
                    th = ths[c]

                    if O["p2_bf16"]:
                        import itertools
                        _btc = itertools.count()
                        def bt(tag="b16"):
                            return sc1.tile([P, F], BF16, tag=tag,
                                            name=f"bt{next(_btc)}")
                        C = lambda k: cov_t[:, :, k]  # noqa: E731
                        b0, b1, b2 = bt("lng1"), bt("lng1"), bt("lng1")
                        nc.gpsimd.tensor_tensor(out=b0, in0=C(3), in1=C(2),
                                                op=OP.subtract)
                        nc.gpsimd.tensor_tensor(out=b1, in0=C(7), in1=C(6),
                                                op=OP.subtract)
                        nc.gpsimd.tensor_tensor(out=b2, in0=C(11), in1=C(10),
                                                op=OP.subtract)
                        e1b = bt()
                        nc.gpsimd.tensor_tensor(out=e1b, in0=C(15), in1=C(11),
                                                op=OP.subtract)
                        dtb = bt()
                        nc.gpsimd.tensor_tensor(out=dtb, in0=e1b, in1=b2,
                                                op=OP.subtract)
                        rd = bt()
                        p2_lns.append(recip(rd, dtb, [P, F]))
                        g0, g1, g2 = bt(), bt(), bt()
                        nc.vector.tensor_tensor(out=g0, in0=b0, in1=rd, op=OP.mult)
                        nc.vector.tensor_tensor(out=g1, in0=b1, in1=rd, op=OP.mult)
                        nc.vector.tensor_tensor(out=g2, in0=b2, in1=rd, op=OP.mult)
                        p00, p01, p02 = bt(), bt(), bt()
                        p11, p12, p22 = bt(), bt(), bt()
                        nc.vector.tensor_tensor(out=p00, in0=g0, in1=b0, op=OP.mult)
                        nc.vector.tensor_tensor(out=p01, in0=g0, in1=b1, op=OP.mult)
                        nc.vector.tensor_tensor(out=p02, in0=g0, in1=b2, op=OP.mult)
                        nc.vector.tensor_tensor(out=p11, in0=g1, in1=b1, op=OP.mult)
                        nc.vector.tensor_tensor(out=p12, in0=g1, in1=b2, op=OP.mult)
                        nc.vector.tensor_tensor(out=p22, in0=g2, in1=b2, op=OP.mult)
                        s00, s01, s02 = bt("lng1"), bt("lng1"), bt("lng1")
                        s11, s12, s22v = bt("lng1"), bt("lng1"), bt("lng1")
                        nc.vector.tensor_tensor(out=s00, in0=C(0), in1=p00,
                                                op=OP.subtract)
                        nc.vector.tensor_tensor(out=s01, in0=C(1), in1=p01,
                                                op=OP.subtract)
                        nc.vector.tensor_tensor(out=s02, in0=C(2), in1=p02,
                                                op=OP.subtract)
                        nc.vector.tensor_tensor(out=s11, in0=C(5), in1=p11,
                                                op=OP.subtract)
                        nc.vector.tensor_tensor(out=s12, in0=C(6), in1=p12,
                                                op=OP.subtract)
                        nc.vector.tensor_tensor(out=s22v, in0=C(10), in1=p22,
                                                op=OP.subtract)
                        r1, r2, r3 = bt("rr1"), bt("rr1"), bt("rr1")
                        p2_lns.append(recip(r1, s00, [P, F]))
                        L21, L31 = bt("lng1"), bt("lng1")
                        nc.vector.tensor_tensor(out=L21, in0=s01, in1=r1, op=OP.mult)
                        nc.vector.tensor_tensor(out=L31, in0=s02, in1=r1, op=OP.mult)
                        pd1, pd2 = bt(), bt()
                        nc.vector.tensor_tensor(out=pd1, in0=L21, in1=s01, op=OP.mult)
                        nc.vector.tensor_tensor(out=pd2, in0=L21, in1=s02, op=OP.mult)
                        D2, m32 = bt("lng1"), bt("lng1")
                        nc.vector.tensor_tensor(out=D2, in0=s11, in1=pd1,
                                                op=OP.subtract)
                        nc.vector.tensor_tensor(out=m32, in0=s12, in1=pd2,
                                                op=OP.subtract)
                        p2_lns.append(recip(r2, D2, [P, F]))
                        l32 = bt("lng1")
                        nc.vector.tensor_tensor(out=l32, in0=m32, in1=r2, op=OP.mult)
                        qa, qb, d3a, d3f = bt(), bt(), bt(), bt()
                        nc.vector.tensor_tensor(out=qa, in0=s02, in1=L31, op=OP.mult)
                        nc.vector.tensor_tensor(out=qb, in0=m32, in1=l32, op=OP.mult)
                        nc.vector.tensor_tensor(out=d3a, in0=s22v, in1=qa,
                                                op=OP.subtract)
                        nc.vector.tensor_tensor(out=d3f, in0=d3a, in1=qb,
                                                op=OP.subtract)
                        p2_lns.append(recip(r3, d3f, [P, F]))
                        pw1, pw2 = bt(), bt()
                        nc.vector.tensor_tensor(out=pw1, in0=L21, in1=wpack,
                                                op=OP.mult)
                        nc.vector.tensor_tensor(out=pw2, in0=L31, in1=wpack,
                                                op=OP.mult)
                        w2t = bt("w2t")
                        nc.vector.tensor_tensor(out=w2t, in0=ytile, in1=pw1,
                                                op=OP.subtract)
                        pw32, ps3 = bt(), bt()
                        nc.vector.tensor_tensor(out=pw32, in0=l32, in1=w2t,
                                                op=OP.mult)
                        nc.vector.tensor_tensor(out=ps3, in0=pw2, in1=pw32,
                                                op=OP.add)
                        w3t = bt("w3t")
                        w3bias = 0.0 if INT_CONV == "rtne" else -16 * PI
                        nc.vector.scalar_tensor_tensor(
                            out=w3t, in0=th, scalar=w3bias, in1=ps3,
                            op0=OP.subtract, op1=OP.subtract)
                        for i, (wt, rt) in enumerate(
                                [(wpack, r1), (w2t, r2), (w3t, r3)]):
                            wsq_i = sc1.tile([P, F], F32, tag="wsqt")
                            nc.scalar.activation(out=wsq_i, in_=wt,
                                                 func=AT.Square)
                            pout = psp.tile([P, F], F32, tag="pout")
                            nc.vector.scalar_tensor_tensor(
                                out=pout, in0=wsq_i, scalar=1.0, in1=rt,
                                op0=OP.mult, op1=OP.mult,
                                accum_out=outacc[:, 3 * c + i : 3 * c + i + 1])
                        continue

                    bpack = sc3.tile([P, F, 3], F32, tag="sc3")
                    nc.gpsimd.tensor_tensor(
                        out=bpack, in0=_sub_ap(cov_t, F, 3, 4, 3),
                        in1=_sub_ap(cov_t, F, 2, 4, 3), op=OP.subtract)
                    e1 = sc1.tile([P, F], F32, tag="sc1")
                    nc.gpsimd.tensor_tensor(
                        out=e1, in0=cov_t[:, :, 15], in1=cov_t[:, :, 11],
                        op=OP.subtract)
                    dt = sc1.tile([P, F], F32, tag="sc1")
                    nc.gpsimd.tensor_tensor(
                        out=dt, in0=e1, in1=bpack[:, :, 2], op=OP.subtract)
                    rdt = sc1.tile([P, F], F32, tag="sc1")
                    p2_lns.append(recip(rdt, dt, [P, F]))
                    gpack = sc3.tile([P, F, 3], F32, tag="sc3")
                    nc.gpsimd.tensor_tensor(
                        out=gpack, in0=bpack, in1=_bc(rdt[:, :], 3), op=OP.mult)
                    pr0 = sc3.tile([P, F, 3], F32, tag="sc3")
                    eng_pr = nc.gpsimd if O["pr01_gps"] else nc.vector
                    eng_pr.tensor_tensor(
                        out=pr0, in0=_bc(gpack[:, :, 0], 3), in1=bpack, op=OP.mult)
                    s0 = sc3.tile([P, F, 3], F32, tag="sc3")
                    nc.vector.tensor_tensor(
                        out=s0, in0=cov_t[:, :, 0:3], in1=pr0, op=OP.subtract)
                    pr1 = sc2.tile([P, F, 2], F32, tag="sc2")
                    eng_pr.tensor_tensor(
                        out=pr1, in0=_bc(gpack[:, :, 1], 2), in1=bpack[:, :, 1:3],
                        op=OP.mult)
                    s1 = sc2.tile([P, F, 2], F32, tag="sc2")
                    nc.vector.tensor_tensor(
                        out=s1, in0=cov_t[:, :, 5:7], in1=pr1, op=OP.subtract)
                    pr2 = sc1.tile([P, F], F32, tag="sc1")
                    nc.vector.tensor_tensor(
                        out=pr2, in0=gpack[:, :, 2], in1=bpack[:, :, 2], op=OP.mult)
                    s22 = sc1.tile([P, F], F32, tag="sc1")
                    nc.vector.tensor_tensor(
                        out=s22, in0=cov_t[:, :, 10], in1=pr2, op=OP.subtract)

                    rpack = sc3.tile([P, F, 3], F32, tag="sc3")
                    recip(rpack[:, :, 0], s0[:, :, 0], [P, F])
                    Lp = sc2.tile([P, F, 2], F32, tag="sc2")  # [L21, L31]
                    nc.vector.tensor_tensor(
                        out=Lp, in0=s0[:, :, 1:3], in1=_bc(rpack[:, :, 0], 2),
                        op=OP.mult)
                    pD = sc2.tile([P, F, 2], F32, tag="sc2")
                    nc.vector.tensor_tensor(
                        out=pD, in0=_bc(Lp[:, :, 0], 2), in1=s0[:, :, 1:3],
                        op=OP.mult)
                    dm = sc2.tile([P, F, 2], F32, tag="sc2")  # [D2, m32]
                    nc.vector.tensor_tensor(
                        out=dm, in0=s1, in1=pD, op=OP.subtract)
                    recip(rpack[:, :, 1], dm[:, :, 0], [P, F])
                    l32 = sc1.tile([P, F], F32, tag="sc1")
                    nc.vector.tensor_tensor(
                        out=l32, in0=dm[:, :, 1], in1=rpack[:, :, 1], op=OP.mult)
                    qa = sc1.tile([P, F], F32, tag="sc1")
                    nc.vector.tensor_tensor(
                        out=qa, in0=s0[:, :, 2], in1=Lp[:, :, 1], op=OP.mult)
                    qb = sc1.tile([P, F], F32, tag="sc1")
                    nc.vector.tensor_tensor(
                        out=qb, in0=dm[:, :, 1], in1=l32, op=OP.mult)
                    d3a = sc1.tile([P, F], F32, tag="sc1")
                    nc.vector.tensor_tensor(
                        out=d3a, in0=s22, in1=qa, op=OP.subtract)
                    d3f = sc1.tile([P, F], F32, tag="sc1")
                    nc.vector.tensor_tensor(
                        out=d3f, in0=d3a, in1=qb, op=OP.subtract)
                    recip(rpack[:, :, 2], d3f, [P, F])

                    pw01 = sc2.tile([P, F, 2], F32, tag="sc2")
                    nc.vector.tensor_tensor(
                        out=pw01, in0=Lp, in1=_bc(wpack[:, :, 0], 2), op=OP.mult)
                    nc.vector.tensor_tensor(
                        out=wpack[:, :, 1], in0=ytile, in1=pw01[:, :, 0],
                        op=OP.subtract)
                    pw32 = sc1.tile([P, F], F32, tag="sc1")
                    nc.vector.tensor_tensor(
                        out=pw32, in0=l32, in1=wpack[:, :, 1], op=OP.mult)
                    ps3 = sc1.tile([P, F], F32, tag="sc1")
                    nc.vector.tensor_tensor(
                        out=ps3, in0=pw01[:, :, 1], in1=pw32, op=OP.add)
                    w3bias = 0.0 if INT_CONV == "rtne" else -16 * PI
                    nc.vector.scalar_tensor_tensor(
                        out=wpack[:, :, 2], in0=th, scalar=w3bias, in1=ps3,
                        op0=OP.subtract, op1=OP.subtract)

                    wsq = sc3.tile([P, F, 3], F32, tag="sc3")
                    if O["wsq_dve"]:
                        nc.vector.tensor_tensor(
                            out=wsq, in0=wpack[:, :, :], in1=wpack[:, :, :],
                            op=OP.mult)
                    else:
                        nc.scalar.activation(out=wsq, in_=wpack[:, :, :],
                                             func=AT.Square)
                    pout = psp.tile([P, F, 3], F32, tag="pout")
                    nc.vector.scalar_tensor_tensor(
                        out=pout, in0=wsq, scalar=1.0, in1=rpack,
                        op0=OP.mult, op1=OP.mult,
                        accum_out=outacc[:, c : c + 1])
                    if O["p3mid"]:
                        emit_p3_v2(sel=c)

                if not (O["p3early"] or O["p3mid"]):
                    emit_p3()

                # keep ACT table sets grouped: all 1/mx exp pairs, then all
                # arctans, then everything ln/exp again (avoids ~4 table
                # reloads x 2.7us per pass)
                for a_i in atan_is:
                    for r_i in rmx_lns:
                        tile.add_dep_helper(a_i.ins, r_i.ins, sync=False,
                                            reason="act set order")
                for l_i in p2_lns:
                    for a_i in atan_is:
                        tile.add_dep_helper(l_i.ins, a_i.ins, sync=False,
                                            reason="act set order")
                nc.sync.dma_start(out=out[:, :], in_=outacc)

            if repeat > 1:
                with tc.For_i(0, repeat, 1):
                    _body()
            else:
                _body()

    if split_waits:
        split_multi_waits(nc)
    return nc, ncols, nqf, nch3


_CACHE = {}


def _get_nc():
    if "nc" not in _CACHE:
        _CACHE["nc"] = build_nc()
    return _CACHE["nc"]


def kernel(q, q_hat, cov, device=0, _return_raw=False):
    nc, ncols, nch, nch3 = _get_nc()
    N = int(np.prod(q.shape[:-1]))
    rows = N // NCORES  # elements per core
    qf = np.ascontiguousarray(np.asarray(q).reshape(N, 4), dtype=np.float32)
    qhf = np.ascontiguousarray(np.asarray(q_hat).reshape(N, 4), dtype=np.float32)
    covf = np.ascontiguousarray(np.asarray(cov).reshape(N, 16), dtype=np.float32)
    in_maps = []
    for k in range(NCORES):
        sl = slice(k * rows, (k + 1) * rows)
        in_maps.append(
            {
                "q": qf[sl].reshape(P, -1),
                "q_hat": qhf[sl].reshape(P, -1),
                "cov": covf[sl].reshape(P, -1),
            }
        )
    res = run_bass_kernel_spmd(nc, in_maps, list(range(NCORES)))
    outs = np.stack([np.asarray(res.results[k]["out"]) for k in range(NCORES)])
    if _return_raw:
        return outs
    S = outs.astype(np.float64)
    qf_sum = S[:, :, 0:nch].sum()
    ln_sum = S[:, :, nch : nch + nch3].sum()
    total = (0.5 * qf_sum + 0.25 * ln_sum) / float(N)
    return np.array(total, dtype=np.float32)



# revision 6
# speedup vs baseline: 1.1792x; 1.1792x over previous
"""Trainium2 Bass kernel for nn_BoxMinusMatNLLLoss.

Reference (per element n of N = B*T = 1024*1024):
    qd   = [x, y, th, th],  x = q0-qh0, y = q1-qh1,
           th = vpi(q2 - atan2(qh2, qh3 + eps*(qh3==0)))
    l_traj = 0.5 * qd^T inv(C) qd          (C symmetric SPD 4x4)
    l_cov  = 0.5 * log(||C||_F) = 0.25 * log(sum(C^2))
    out = mean(l_traj) + mean(l_cov)       (scalar f32)

Kernel design (pure data parallel, 8 cores; per core 131072 elements as
[128 partitions, E=1024]):

  * atan2 by range reduction: r = min(|y|,|x|)/max(|y|,|x|) in [0,1]
    (the ACT Arctan table only covers [-pi/2, pi/2]), and since the
    downstream wrap works mod 2pi:
        atan2(y,x) == sign(x*y)*phi + pi*(x<0)   (mod 2pi),
        phi = |swap*pi/2 - arctan(r)|.
  * wrap: th = delta - 2pi*round(delta/(2pi)) via f32->int32 copy
    (hardware rounds RTNE; the CoreSim truncates, so sims build the
    "trunc" variant with a +8.5 offset).
  * qd = [x,y,t,t] Schur-reduces the 4x4 solve to a 3x3 LDL^T:
        QF = u^T S^-1 u, u = (x,y,t), S = A - b b^T/dt,
        b_i = C[i,3]-C[i,2], dt = C22+C33-2*C23.
  * All reciprocals (5/element, all positive arguments) run on the ACT
    engine as exp(-ln(x)) — Ln and Exp share one table set, and the DVE
    InstReciprocal measures ~20x slower than an elementwise op.
  * l_cov: sum(C^2) = sum(diag^2) + 2 sum(offdiag^2) via ACT Square
    (scale=sqrt(2) folds the 2), segmented-reduced on GPSIMD (tree adds)
    and DVE (tensor_reduce), then ACT Ln with accum_out.
  * Partial sums leave the chip as a [128, nchunk-ish] tile per core via
    accum_out; the host sums 8 small tiles and applies 0.5/N, 0.25/N.
"""

import math

import numpy as np

import concourse.bass as bass
import concourse.tile as tile
from concourse import mybir
from concourse.bass_utils import run_bass_kernel_spmd

F32 = mybir.dt.float32
I32 = mybir.dt.int32
BF16 = mybir.dt.bfloat16
PI = math.pi
NCORES = 8
P = 128

# f32 -> int32 copy semantics: hardware = "rtne" (probe-verified);
# CoreSim = "trunc".
INT_CONV = "rtne"
# "act" = exp(-ln x) pairs on ScalarE; "dve" = InstReciprocal (slow).
RECIP_MODE = "act"


def _sub_ap(t, f_count, inner_off, inner_step, inner_cnt):
    """[P, f_count, inner_cnt] view of a [P, f_count, K] tile with custom
    inner offset/step (e.g. matrix diagonals)."""
    a = t[:, :, :]
    return bass.AP(
        tensor=a.tensor,
        offset=a.offset + inner_off * a.ap[2][0],
        ap=[a.ap[0], [a.ap[1][0], f_count], [a.ap[2][0] * inner_step, inner_cnt]],
    )


def _bc(ap2d, n):
    """Broadcast a [P, F] AP to [P, F, n] with step 0 on the inner dim."""
    return bass.AP(
        tensor=ap2d.tensor,
        offset=ap2d.offset,
        ap=[ap2d.ap[0], ap2d.ap[1], [0, n]],
    )


def split_multi_waits(nc):
    """The walrus build in this container encodes only one sync wait per
    instruction; Tile's tail drain carries several.  Split extras into
    single-wait NOPs placed just before."""
    for fn in nc.m.functions:
        for bb in fn.blocks:
            new_insts = []
            for ins in bb.instructions:
                si = ins.sync_info
                if si is not None and si.on_wait and len(si.on_wait) > 1:
                    waits = list(si.on_wait)
                    for w in waits[:-1]:
                        nop = mybir.InstNoOp(
                            name=nc.get_next_instruction_name(), ins=[], outs=[]
                        )
                        nop.engine = ins.engine
                        nop.sync_info = mybir.SyncInfo(on_wait=[w], on_update=[])
                        new_insts.append(nop)
                    si.on_wait = [waits[-1]]
                new_insts.append(ins)
            bb.instructions = new_insts


def build_nc(E=1024, F=512, F3=256, gps_tree_subchunks=(0, 1, 2),
             split_waits=True, bufs=None, repeat=1, opts=None):
    """Build the per-core Bass program (see module docstring)."""
    assert E % F == 0 and E % F3 == 0
    nch = E // F
    nch3 = E // F3
    nqf_per = 3 if (opts or {}).get("p2_bf16") else 1
    nqf = nch * nqf_per
    ncols = nqf + nch3  # qf partials + ln partials
    AT = mybir.ActivationFunctionType
    OP = mybir.AluOpType
    AX = mybir.AxisListType

    nc = bass.Bass()
    q = nc.declare_dram_parameter("q", [P, E * 4], F32, isOutput=False)
    qh = nc.declare_dram_parameter("q_hat", [P, E * 4], F32, isOutput=False)
    cov = nc.declare_dram_parameter("cov", [P, E * 16], F32, isOutput=False)
    out = nc.declare_dram_parameter("out", [P, ncols], F32, isOutput=True)

    qv = q.rearrange("p (e c) -> p e c", c=4)
    qhv = qh.rearrange("p (e c) -> p e c", c=4)
    covv = cov.rearrange("p (e c) -> p e c", c=16)

    B = dict(inp=2, inq=1, sc1=6, rc=4, p3s=4, sc2=4, sc3=4, long=2,
             p3=2, p31=1, acc=1, psum=2)
    if bufs:
        B.update(bufs)
    O = dict(abs_dve=False, sgm_dve=False, pr01_gps=False,
             skip_p1=False, skip_p2=False, skip_p3=False, p2_bf16=False,
             wsq_dve=False, p3sq_dve=False, phi_dve=False, p3early=False,
             p3v2=False, p3mid=False)
    if opts:
        O.update(opts)
    with tile.TileContext(nc) as tc:
        with (
            tc.tile_pool(name="inp", bufs=B["inp"]) as inp,
            tc.tile_pool(name="inq", bufs=B["inq"]) as inq,
            tc.tile_pool(name="sc1", bufs=B["sc1"]) as sc1,
            tc.tile_pool(name="rc", bufs=B["rc"]) as rcp,
            tc.tile_pool(name="p3s", bufs=B["p3s"]) as p3sp,
            tc.tile_pool(name="sc2", bufs=B["sc2"]) as sc2,
            tc.tile_pool(name="sc3", bufs=B["sc3"]) as sc3,
            tc.tile_pool(name="long", bufs=B["long"]) as lng,
            tc.tile_pool(name="p3", bufs=B["p3"]) as p3p,
            tc.tile_pool(name="p31", bufs=B["p31"]) as p31,
            tc.tile_pool(name="acc", bufs=B["acc"]) as accp,
            tc.tile_pool(name="psum", bufs=B["psum"], space="PSUM") as psp,
        ):
            def _body():
                outacc = accp.tile([P, ncols], F32)
                nc.vector.memset(outacc, 0.0)

                def recip(out_ap, in_ap, shape):
                    if RECIP_MODE == "dve":
                        return nc.vector.reciprocal(out=out_ap, in_=in_ap)
                    lnt = rcp.tile(shape, F32, tag="rc")
                    ln_i = nc.scalar.activation(out=lnt, in_=in_ap, func=AT.Ln)
                    nc.scalar.activation(out=out_ap, in_=lnt, func=AT.Exp,
                                         scale=-1.0)
                    return ln_i

                q_ts, qh_ts, cov_ts = [], [], []
                for c in range(nch):
                    lo = c * F
                    q_t = inq.tile([P, F, 4], F32, tag="q")
                    qh_t = inq.tile([P, F, 4], F32, tag="qh")
                    nc.sync.dma_start(out=q_t, in_=qv[:, lo : lo + F, :])
                    nc.sync.dma_start(out=qh_t, in_=qhv[:, lo : lo + F, :])
                    cov_t = inp.tile([P, F, 16], F32, tag="cov")
                    nc.sync.dma_start(out=cov_t, in_=covv[:, lo : lo + F, :])
                    q_ts.append(q_t)
                    qh_ts.append(qh_t)
                    cov_ts.append(cov_t)

                # ------- P1a: |.|, min/max, swap flag, 1/mx, r  (ln/exp set)
                p1 = []
                rmx_lns, atan_is, p2_lns = [], [], []
                for c in range(nch):
                    q_t, qh_t = q_ts[c], qh_ts[c]
                    if O["p2_bf16"]:
                        wpack = lng.tile([P, F], BF16, tag="wpack")  # x
                        xslot = wpack
                    else:
                        wpack = lng.tile([P, F, 3], F32, tag="wpack")
                        xslot = wpack[:, :, 0]
                    ytile = lng.tile([P, F], BF16 if O["p2_bf16"] else F32,
                                     tag="ytile")
                    nc.gpsimd.tensor_tensor(
                        out=xslot, in0=q_t[:, :, 0], in1=qh_t[:, :, 0],
                        op=OP.subtract)
                    nc.gpsimd.tensor_tensor(
                        out=ytile, in0=q_t[:, :, 1], in1=qh_t[:, :, 1],
                        op=OP.subtract)
                    ay = sc1.tile([P, F], F32, tag="sc1")
                    nc.scalar.activation(out=ay, in_=qh_t[:, :, 2], func=AT.Abs)
                    ax = sc1.tile([P, F], F32, tag="sc1")
                    nc.scalar.activation(out=ax, in_=qh_t[:, :, 3], func=AT.Abs)
                    mx = sc1.tile([P, F], F32, tag="sc1")
                    nc.vector.tensor_tensor(out=mx, in0=ax, in1=ay, op=OP.max)
                    mn = sc1.tile([P, F], F32, tag="sc1")
                    nc.vector.tensor_tensor(out=mn, in0=ax, in1=ay, op=OP.min)
                    sw = lng.tile([P, F], F32, tag="sw")
                    nc.vector.tensor_tensor(out=sw, in0=ay, in1=ax, op=OP.is_gt)
                    rmx = sc1.tile([P, F], F32, tag="sc1")
                    rmx_lns.append(recip(rmx, mx, [P, F]))
                    rr = lng.tile([P, F], F32, tag="rr")
                    nc.vector.tensor_tensor(out=rr, in0=mn, in1=rmx, op=OP.mult)
                    xy = lng.tile([P, F], F32, tag="xy")
                    nc.vector.tensor_tensor(
                        out=xy, in0=qh_t[:, :, 2], in1=qh_t[:, :, 3], op=OP.mult)
                    neg = sc1.tile([P, F], F32, tag="sc1")
                    nc.vector.tensor_scalar(
                        out=neg, in0=qh_t[:, :, 3], scalar1=0.0, scalar2=None,
                        op0=OP.is_lt)
                    base = lng.tile([P, F], F32, tag="base")
                    nc.vector.scalar_tensor_tensor(
                        out=base, in0=neg, scalar=-PI, in1=q_t[:, :, 2],
                        op0=OP.mult, op1=OP.add)
                    p1.append((wpack, ytile, sw, rr, xy, base))

                def emit_p3_v2(sel=None):
                    sub_per_chunk = F // F3
                    for s in range(nch3 if not O["skip_p3"] else 0):
                        c = s // sub_per_chunk
                        if sel is not None and c != sel:
                            continue
                        off = (s % sub_per_chunk) * F3
                        cov_t = cov_ts[c]
                        # component-major: sq10v[:, k*F3:(k+1)*F3] = comp k
                        sq10v = p3p.tile([P, 10 * F3], F32, tag="sq10")
                        cs = cov_t[:, :, :]

                        def cmaj(o, st, n):
                            # in: comps o, o+st, ... (n comps) over F3 elements
                            return bass.AP(
                                tensor=cs.tensor,
                                offset=cs.offset + (off * 16 + o),
                                ap=[cs.ap[0], [16, F3], [st, n]],
                            )

                        def omaj(k0, n):
                            a = sq10v[:, :]
                            return bass.AP(
                                tensor=a.tensor,
                                offset=a.offset + k0 * F3,
                                ap=[a.ap[0], [1, F3], [F3, n]],
                            )

                        r2s = math.sqrt(2.0)
                        nc.scalar.activation(
                            out=omaj(0, 4), in_=cmaj(0, 5, 4), func=AT.Square)
                        nc.scalar.activation(
                            out=omaj(4, 3), in_=cmaj(1, 1, 3), func=AT.Square,
                            scale=r2s)
                        nc.scalar.activation(
                            out=omaj(7, 2), in_=cmaj(6, 1, 2), func=AT.Square,
                            scale=r2s)
                        nc.scalar.activation(
                            out=omaj(9, 1), in_=cmaj(11, 1, 1), func=AT.Square,
                            scale=r2s)
                        t5 = p31.tile([P, 5 * F3], F32, tag="t5")
                        nc.gpsimd.tensor_tensor(
                            out=t5, in0=sq10v[:, 0 : 5 * F3],
                            in1=sq10v[:, 5 * F3 : 10 * F3], op=OP.add)
                        t2t = p31.tile([P, 2 * F3], F32, tag="t2t")
                        nc.gpsimd.tensor_tensor(
                            out=t2t, in0=t5[:, 0 : 2 * F3],
                            in1=t5[:, 2 * F3 : 4 * F3], op=OP.add)
                        sa = p3sp.tile([P, F3], F32, tag="p3s")
                        nc.gpsimd.tensor_tensor(
                            out=sa, in0=t2t[:, 0:F3], in1=t2t[:, F3 : 2 * F3],
                            op=OP.add)
                        ssq = p3sp.tile([P, F3], F32, tag="p3s")
                        nc.gpsimd.tensor_tensor(
                            out=ssq, in0=sa, in1=t5[:, 4 * F3 : 5 * F3],
                            op=OP.add)
                        lnsc = p3sp.tile([P, F3], F32, tag="p3s")
                        p2_lns.append(nc.scalar.activation(
                            out=lnsc, in_=ssq, func=AT.Ln,
                            accum_out=outacc[:, nqf + s : nqf + s + 1]))

                def emit_p3():
                    if O["p3v2"]:
                        return emit_p3_v2()
                    # ------- P3: l_cov  (ln/exp set)
                    sub_per_chunk = F // F3
                    for s in range(nch3 if not O["skip_p3"] else 0):
                        c = s // sub_per_chunk
                        off = (s % sub_per_chunk) * F3
                        cov_t = cov_ts[c]
                        sq10 = p3p.tile([P, F3, 10], F32, tag="sq10")
                        cs = cov_t[:, :, :]

                        def cslice(o, st, n):
                            return bass.AP(
                                tensor=cs.tensor,
                                offset=cs.offset + (off * 16 + o),
                                ap=[cs.ap[0], [16, F3], [st, n]],
                            )

                        r2s = math.sqrt(2.0)
                        nc.scalar.activation(
                            out=sq10[:, :, 0:4], in_=cslice(0, 5, 4), func=AT.Square)
                        nc.scalar.activation(
                            out=sq10[:, :, 7:9], in_=cslice(6, 1, 2), func=AT.Square,
                            scale=r2s)
                        if O["p3sq_dve"]:
                            # 2*c^2 via one STT: (c*2) mult c
                            nc.vector.scalar_tensor_tensor(
                                out=sq10[:, :, 4:7], in0=cslice(1, 1, 3), scalar=2.0,
                                in1=cslice(1, 1, 3), op0=OP.mult, op1=OP.mult)
                            nc.vector.scalar_tensor_tensor(
                                out=sq10[:, :, 9:10], in0=cslice(11, 1, 1), scalar=2.0,
                                in1=cslice(11, 1, 1), op0=OP.mult, op1=OP.mult)
                        else:
                            nc.scalar.activation(
                                out=sq10[:, :, 4:7], in_=cslice(1, 1, 3),
                                func=AT.Square, scale=r2s)
                            nc.scalar.activation(
                                out=sq10[:, :, 9:10], in_=cslice(11, 1, 1),
                                func=AT.Square, scale=r2s)
                        ssq = p3sp.tile([P, F3], F32, tag="p3s")
                        if (s % nch3) in gps_tree_subchunks:
                            t5 = p31.tile([P, F3, 5], F32, tag="t5")
                            nc.gpsimd.tensor_tensor(
                                out=t5, in0=sq10[:, :, 0:5], in1=sq10[:, :, 5:10],
                                op=OP.add)
                            t2t = p31.tile([P, F3, 2], F32, tag="t2t")
                            nc.gpsimd.tensor_tensor(
                                out=t2t, in0=t5[:, :, 0:2], in1=t5[:, :, 2:4], op=OP.add)
                            sa = p3sp.tile([P, F3], F32, tag="p3s")
                            nc.gpsimd.tensor_tensor(
                                out=sa, in0=t2t[:, :, 0], in1=t2t[:, :, 1], op=OP.add)
                            nc.gpsimd.tensor_tensor(
                                out=ssq, in0=sa, in1=t5[:, :, 4], op=OP.add)
                        else:
                            nc.vector.tensor_reduce(
                                out=ssq, in_=sq10, axis=AX.X, op=OP.add)
                        lnsc = p3sp.tile([P, F3], F32, tag="p3s")
                        p2_lns.append(nc.scalar.activation(
                            out=lnsc, in_=ssq, func=AT.Ln,
                            accum_out=outacc[:, nqf + s : nqf + s + 1]))


                if O["p3early"]:
                    emit_p3()

                # ------- P1b: arctan (trig set), quadrant fold, wrap
                ths = []
                for c in range(nch):
                    wpack, ytile, sw, rr, xy, base = p1[c]
                    if O["skip_p1"]:
                        ths.append(base)
                        continue
                    at = sc1.tile([P, F], F32, tag="sc1")
                    atan_is.append(
                        nc.scalar.activation(out=at, in_=rr, func=AT.Arctan))
                    t1 = sc1.tile([P, F], F32, tag="sc1")
                    nc.vector.scalar_tensor_tensor(
                        out=t1, in0=sw, scalar=PI / 2, in1=at,
                        op0=OP.mult, op1=OP.subtract)
                    phi = sc1.tile([P, F], F32, tag="sc1")
                    if O["phi_dve"]:
                        nc.vector.tensor_scalar(
                            out=phi.bitcast(I32), in0=t1.bitcast(I32),
                            scalar1=0x7FFFFFFF, scalar2=None, op0=OP.bitwise_and)
                    else:
                        nc.scalar.activation(out=phi, in_=t1, func=AT.Abs)
                    sgm = sc1.tile([P, F], F32, tag="sc1")
                    if O["sgm_dve"]:
                        nc.vector.tensor_scalar(
                            out=sgm, in0=xy, scalar1=0.0, scalar2=2.0,
                            op0=OP.is_ge, op1=OP.mult)
                        nc.vector.tensor_scalar(
                            out=sgm, in0=sgm, scalar1=1.0, scalar2=None,
                            op0=OP.subtract)
                    else:
                        nc.scalar.activation(out=sgm, in_=xy, func=AT.Sign)
                    mp = sc1.tile([P, F], F32, tag="sc1")
                    nc.vector.tensor_tensor(out=mp, in0=sgm, in1=phi, op=OP.mult)
                    delta = sc1.tile([P, F], F32, tag="sc1")
                    nc.vector.tensor_tensor(
                        out=delta, in0=base, in1=mp, op=OP.subtract)
                    # th = delta - 2pi*round(delta/2pi); trunc variant offsets
                    # by +8.5 (z>0 so trunc==floor) and folds -16pi into w3.
                    zbias = 0.0 if INT_CONV == "rtne" else 8.5
                    z = sc1.tile([P, F], F32, tag="sc1")
                    nc.vector.tensor_scalar(
                        out=z, in0=delta, scalar1=1.0 / (2 * PI), scalar2=zbias,
                        op0=OP.mult, op1=OP.add)
                    fi = sc1.tile([P, F], I32, tag="sc1")
                    nc.vector.tensor_copy(fi, z)
                    ff = sc1.tile([P, F], F32, tag="sc1")
                    nc.vector.tensor_copy(ff, fi)
                    th = lng.tile([P, F], BF16 if O["p2_bf16"] else F32,
                                  tag="th")
                    nc.vector.scalar_tensor_tensor(
                        out=th, in0=ff, scalar=-2 * PI, in1=delta,
                        op0=OP.mult, op1=OP.add)
                    ths.append(th)

                # ------- P2: Schur + LDL + forward + QF  (ln/exp set)
                for c in range(nch):
                    if O["skip_p2"]:
                        break
                    cov_t = cov_ts[c]
                    wpack, ytile = p1[c][0], p1[c][1]
                    th = ths[c]

                    if O["p2_bf16"]:
                        import itertools
                        _btc = itertools.count()
                        def bt(tag="b16"):
                            return sc1.tile([P, F], BF16, tag=tag,
                                            name=f"bt{next(_btc)}")
                        C = lambda k: cov_t[:, :, k]  # noqa: E731
                        b0, b1, b2 = bt("lng1"), bt("lng1"), bt("lng1")
                        nc.gpsimd.tensor_tensor(out=b0, in0=C(3), in1=C(2),
                                                op=OP.subtract)
                        nc.gpsimd.tensor_tensor(out=b1, in0=C(7), in1=C(6),
                                                op=OP.subtract)
                        nc.gpsimd.tensor_tensor(out=b2, in0=C(11), in1=C(10),
                                                op=OP.subtract)
                        e1b = bt()
                        nc.gpsimd.tensor_tensor(out=e1b, in0=C(15), in1=C(11),
                                                op=OP.subtract)
                        dtb = bt()
                        nc.gpsimd.tensor_tensor(out=dtb, in0=e1b, in1=b2,
                                                op=OP.subtract)
                        rd = bt()
                        p2_lns.append(recip(rd, dtb, [P, F]))
                        g0, g1, g2 = bt(), bt(), bt()
                        nc.vector.tensor_tensor(out=g0, in0=b0, in1=rd, op=OP.mult)
                        nc.vector.tensor_tensor(out=g1, in0=b1, in1=rd, op=OP.mult)
                        nc.vector.tensor_tensor(out=g2, in0=b2, in1=rd, op=OP.mult)
                        p00, p01, p02 = bt(), bt(), bt()
                        p11, p12, p22 = bt(), bt(), bt()
                        nc.vector.tensor_tensor(out=p00, in0=g0, in1=b0, op=OP.mult)
                        nc.vector.tensor_tensor(out=p01, in0=g0, in1=b1, op=OP.mult)
                        nc.vector.tensor_tensor(out=p02, in0=g0, in1=b2, op=OP.mult)
                        nc.vector.tensor_tensor(out=p11, in0=g1, in1=b1, op=OP.mult)
                        nc.vector.tensor_tensor(out=p12, in0=g1, in1=b2, op=OP.mult)
                        nc.vector.tensor_tensor(out=p22, in0=g2, in1=b2, op=OP.mult)
                        s00, s01, s02 = bt("lng1"), bt("lng1"), bt("lng1")
                        s11, s12, s22v = bt("lng1"), bt("lng1"), bt("lng1")
                        nc.vector.tensor_tensor(out=s00, in0=C(0), in1=p00,
                                                op=OP.subtract)
                        nc.vector.tensor_tensor(out=s01, in0=C(1), in1=p01,
                                                op=OP.subtract)
                        nc.vector.tensor_tensor(out=s02, in0=C(2), in1=p02,
                                                op=OP.subtract)
                        nc.vector.tensor_tensor(out=s11, in0=C(5), in1=p11,
                                                op=OP.subtract)
                        nc.vector.tensor_tensor(out=s12, in0=C(6), in1=p12,
                                                op=OP.subtract)
                        nc.vector.tensor_tensor(out=s22v, in0=C(10), in1=p22,
                                                op=OP.subtract)
                        r1, r2, r3 = bt("rr1"), bt("rr1"), bt("rr1")
                        p2_lns.append(recip(r1, s00, [P, F]))
                        L21, L31 = bt("lng1"), bt("lng1")
                        nc.vector.tensor_tensor(out=L21, in0=s01, in1=r1, op=OP.mult)
                        nc.vector.tensor_tensor(out=L31, in0=s02, in1=r1, op=OP.mult)
                        pd1, pd2 = bt(), bt()
                        nc.vector.tensor_tensor(out=pd1, in0=L21, in1=s01, op=OP.mult)
                        nc.vector.tensor_tensor(out=pd2, in0=L21, in1=s02, op=OP.mult)
                        D2, m32 = bt("lng1"), bt("lng1")
                        nc.vector.tensor_tensor(out=D2, in0=s11, in1=pd1,
                                                op=OP.subtract)
                        nc.vector.tensor_tensor(out=m32, in0=s12, in1=pd2,
                                                op=OP.subtract)
                        p2_lns.append(recip(r2, D2, [P, F]))
                        l32 = bt("lng1")
                        nc.vector.tensor_tensor(out=l32, in0=m32, in1=r2, op=OP.mult)
                        qa, qb, d3a, d3f = bt(), bt(), bt(), bt()
                        nc.vector.tensor_tensor(out=qa, in0=s02, in1=L31, op=OP.mult)
                        nc.vector.tensor_tensor(out=qb, in0=m32, in1=l32, op=OP.mult)
                        nc.vector.tensor_tensor(out=d3a, in0=s22v, in1=qa,
                                                op=OP.subtract)
                        nc.vector.tensor_tensor(out=d3f, in0=d3a, in1=qb,
                                                op=OP.subtract)
                        p2_lns.append(recip(r3, d3f, [P, F]))
                        pw1, pw2 = bt(), bt()
                        nc.vector.tensor_tensor(out=pw1, in0=L21, in1=wpack,
                                                op=OP.mult)
                        nc.vector.tensor_tensor(out=pw2, in0=L31, in1=wpack,
                                                op=OP.mult)
                        w2t = bt("w2t")
                        nc.vector.tensor_tensor(out=w2t, in0=ytile, in1=pw1,
                                                op=OP.subtract)
                        pw32, ps3 = bt(), bt()
                        nc.vector.tensor_tensor(out=pw32, in0=l32, in1=w2t,
                                                op=OP.mult)
                        nc.vector.tensor_tensor(out=ps3, in0=pw2, in1=pw32,
                                                op=OP.add)
                        w3t = bt("w3t")
                        w3bias = 0.0 if INT_CONV == "rtne" else -16 * PI
                        nc.vector.scalar_tensor_tensor(
                            out=w3t, in0=th, scalar=w3bias, in1=ps3,
                            op0=OP.subtract, op1=OP.subtract)
                        for i, (wt, rt) in enumerate(
                                [(wpack, r1), (w2t, r2), (w3t, r3)]):
                            wsq_i = sc1.tile([P, F], F32, tag="wsqt")
                            nc.scalar.activation(out=wsq_i, in_=wt,
                                                 func=AT.Square)
                            pout = psp.tile([P, F], F32, tag="pout")
                            nc.vector.scalar_tensor_tensor(
                                out=pout, in0=wsq_i, scalar=1.0, in1=rt,
                                op0=OP.mult, op1=OP.mult,
                                accum_out=outacc[:, 3 * c + i : 3 * c + i + 1])
                        continue

                    bpack = sc3.tile([P, F, 3], F32, tag="sc3")
                    nc.gpsimd.tensor_tensor(
                        out=bpack, in0=_sub_ap(cov_t, F, 3, 4, 3),
                        in1=_sub_ap(cov_t, F, 2, 4, 3), op=OP.subtract)
                    e1 = sc1.tile([P, F], F32, tag="sc1")
                    nc.gpsimd.tensor_tensor(
                        out=e1, in0=cov_t[:, :, 15], in1=cov_t[:, :, 11],
                        op=OP.subtract)
                    dt = sc1.tile([P, F], F32, tag="sc1")
                    nc.gpsimd.tensor_tensor(
                        out=dt, in0=e1, in1=bpack[:, :, 2], op=OP.subtract)
                    rdt = sc1.tile([P, F], F32, tag="sc1")
                    p2_lns.append(recip(rdt, dt, [P, F]))
                    gpack = sc3.tile([P, F, 3], F32, tag="sc3")
                    nc.gpsimd.tensor_tensor(
                        out=gpack, in0=bpack, in1=_bc(rdt[:, :], 3), op=OP.mult)
                    pr0 = sc3.tile([P, F, 3], F32, tag="sc3")
                    eng_pr = nc.gpsimd if O["pr01_gps"] else nc.vector
                    eng_pr.tensor_tensor(
                        out=pr0, in0=_bc(gpack[:, :, 0], 3), in1=bpack, op=OP.mult)
                    s0 = sc3.tile([P, F, 3], F32, tag="sc3")
                    nc.vector.tensor_tensor(
                        out=s0, in0=cov_t[:, :, 0:3], in1=pr0, op=OP.subtract)
                    pr1 = sc2.tile([P, F, 2], F32, tag="sc2")
                    eng_pr.tensor_tensor(
                        out=pr1, in0=_bc(gpack[:, :, 1], 2), in1=bpack[:, :, 1:3],
                        op=OP.mult)
                    s1 = sc2.tile([P, F, 2], F32, tag="sc2")
                    nc.vector.tensor_tensor(
                        out=s1, in0=cov_t[:, :, 5:7], in1=pr1, op=OP.subtract)
                    pr2 = sc1.tile([P, F], F32, tag="sc1")
                    nc.vector.tensor_tensor(
                        out=pr2, in0=gpack[:, :, 2], in1=bpack[:, :, 2], op=OP.mult)
                    s22 = sc1.tile([P, F], F32, tag="sc1")
                    nc.vector.tensor_tensor(
                        out=s22, in0=cov_t[:, :, 10], in1=pr2, op=OP.subtract)

                    rpack = sc3.tile([P, F, 3], F32, tag="sc3")
                    recip(rpack[:, :, 0], s0[:, :, 0], [P, F])
                    Lp = sc2.tile([P, F, 2], F32, tag="sc2")  # [L21, L31]
                    nc.vector.tensor_tensor(
                        out=Lp, in0=s0[:, :, 1:3], in1=_bc(rpack[:, :, 0], 2),
                        op=OP.mult)
                    pD = sc2.tile([P, F, 2], F32, tag="sc2")
                    nc.vector.tensor_tensor(
                        out=pD, in0=_bc(Lp[:, :, 0], 2), in1=s0[:, :, 1:3],
                        op=OP.mult)
                    dm = sc2.tile([P, F, 2], F32, tag="sc2")  # [D2, m32]
                    nc.vector.tensor_tensor(
                        out=dm, in0=s1, in1=pD, op=OP.subtract)
                    recip(rpack[:, :, 1], dm[:, :, 0], [P, F])
                    l32 = sc1.tile([P, F], F32, tag="sc1")
                    nc.vector.tensor_tensor(
                        out=l32, in0=dm[:, :, 1], in1=rpack[:, :, 1], op=OP.mult)
                    qa = sc1.tile([P, F], F32, tag="sc1")
                    nc.vector.tensor_tensor(
                        out=qa, in0=s0[:, :, 2], in1=Lp[:, :, 1], op=OP.mult)
                    qb = sc1.tile([P, F], F32, tag="sc1")
                    nc.vector.tensor_tensor(
                        out=qb, in0=dm[:, :, 1], in1=l32, op=OP.mult)
                    d3a = sc1.tile([P, F], F32, tag="sc1")
                    nc.vector.tensor_tensor(
                        out=d3a, in0=s22, in1=qa, op=OP.subtract)
                    d3f = sc1.tile([P, F], F32, tag="sc1")
                    nc.vector.tensor_tensor(
                        out=d3f, in0=d3a, in1=qb, op=OP.subtract)
                    recip(rpack[:, :, 2], d3f, [P, F])

                    pw01 = sc2.tile([P, F, 2], F32, tag="sc2")
                    nc.vector.tensor_tensor(
                        out=pw01, in0=Lp, in1=_bc(wpack[:, :, 0], 2), op=OP.mult)
                    nc.vector.tensor_tensor(
                        out=wpack[:, :, 1], in0=ytile, in1=pw01[:, :, 0],
                        op=OP.subtract)
                    pw32 = sc1.tile([P, F], F32, tag="sc1")
                    nc.vector.tensor_tensor(
                        out=pw32, in0=l32, in1=wpack[:, :, 1], op=OP.mult)
                    ps3 = sc1.tile([P, F], F32, tag="sc1")
                    nc.vector.tensor_tensor(
                        out=ps3, in0=pw01[:, :, 1], in1=pw32, op=OP.add)
                    w3bias = 0.0 if INT_CONV == "rtne" else -16 * PI
                    nc.vector.scalar_tensor_tensor(
                        out=wpack[:, :, 2], in0=th, scalar=w3bias, in1=ps3,
                        op0=OP.subtract, op1=OP.subtract)

                    wsq = sc3.tile([P, F, 3], F32, tag="sc3")
                    if O["wsq_dve"]:
                        nc.vector.tensor_tensor(
                            out=wsq, in0=wpack[:, :, :], in1=wpack[:, :, :],
                            op=OP.mult)
                    else:
                        nc.scalar.activation(out=wsq, in_=wpack[:, :, :],
                                             func=AT.Square)
                    pout = psp.tile([P, F, 3], F32, tag="pout")
                    nc.vector.scalar_tensor_tensor(
                        out=pout, in0=wsq, scalar=1.0, in1=rpack,
                        op0=OP.mult, op1=OP.mult,
                        accum_out=outacc[:, c : c + 1])
                    if O["p3mid"]:
                        emit_p3_v2(sel=c)

                if not (O["p3early"] or O["p3mid"]):
                    emit_p3()

                # keep ACT table sets grouped: all 1/mx exp pairs, then all
                # arctans, then everything ln/exp again (avoids ~4 table
                # reloads x 2.7us per pass)
                for a_i in atan_is:
                    for r_i in rmx_lns:
                        tile.add_dep_helper(a_i.ins, r_i.ins, sync=False,
                                            reason="act set order")
                for l_i in p2_lns:
                    for a_i in atan_is:
                        tile.add_dep_helper(l_i.ins, a_i.ins, sync=False,
                                            reason="act set order")
                nc.sync.dma_start(out=out[:, :], in_=outacc)

            if repeat > 1:
                with tc.For_i(0, repeat, 1):
                    _body()
            else:
                _body()

    if split_waits:
        split_multi_waits(nc)
    return nc, ncols, nqf, nch3


def _act_recip(nc, out_ap, in_ap):
    """InstActivation func=Reciprocal, emitted directly.

    bass.py guards this func behind a blanket accuracy warning; measured
    on this hardware the table reciprocal is ~1e-5 relative (f32), well
    inside this kernel's 2e-2 budget.  Reciprocal keeps float bias.
    """
    eng = nc.scalar
    inputs = [eng.lower_ap(in_ap)]
    for val in (0.0, 1.0, 0.0):  # bias, scale, alpha
        inputs.append(mybir.ImmediateValue(dtype=mybir.dt.float32, value=val))
    return eng.add_instruction(
        mybir.InstActivation(
            name=nc.get_next_instruction_name(),
            func=mybir.ActivationFunctionType.Reciprocal,
            ins=inputs, outs=[eng.lower_ap(out_ap)],
        )
    )


def build_nc_v2(E=1024, F=512, split_waits=True, repeat=1, opts=None):
    """v2: element-major bf16 pipeline.

    Per element: th via signed 1/x (ACT Reciprocal table, ~1e-5 rel) +
    saturating Arctan — atan2(y,x) == arctan(y/x) + pi*(x<0) (mod 2pi),
    so no range reduction / min/max / quadrant folding is needed.  Wrap
    via fused f32->int32 tensor_scalar (RTNE on hw).  Schur + LDL^T in
    bf16 split tiles ([P,F] contiguous => DVE 2x mode); cov-facing
    subtractions on GPSIMD reading element-major component runs.  l_cov
    squares: diag on ACT (Square, stride-5), off-diag on DVE via
    STT (c*2)*c; comp-packed bf16 adder tree on DVE (2x).
    """
    assert E % F == 0
    nch = E // F
    ncols = 4 * nch  # 3 qf partial cols + 1 ln col per chunk
    AT = mybir.ActivationFunctionType
    OP = mybir.AluOpType

    O = dict(sq_diag_act=True, pool_s01=True, wsq_act=False, w1sq_pool=False,
             tree_pool_t2=False, act_hints=False)
    if opts:
        O.update(opts)

    nc = bass.Bass()
    q = nc.declare_dram_parameter("q", [P, E * 4], F32, isOutput=False)
    qh = nc.declare_dram_parameter("q_hat", [P, E * 4], F32, isOutput=False)
    cov = nc.declare_dram_parameter("cov", [P, E * 16], F32, isOutput=False)
    out = nc.declare_dram_parameter("out", [P, ncols], F32, isOutput=True)

    qv = q.rearrange("p (e c) -> p e c", c=4)
    qhv = qh.rearrange("p (e c) -> p e c", c=4)
    covv = cov.rearrange("p (e c) -> p e c", c=16)

    import itertools
    _tc_counter = itertools.count()
    with tile.TileContext(nc) as tc:
        with (
            tc.tile_pool(name="inq", bufs=2) as inq,
            tc.tile_pool(name="inp", bufs=2) as inp,
            tc.tile_pool(name="w1", bufs=2) as w1p,   # f32 named
            tc.tile_pool(name="s1f", bufs=4) as s1fp,  # f32 rotating scratch
            tc.tile_pool(name="wb", bufs=2) as wbp,   # bf16 named tiles
            tc.tile_pool(name="sb1", bufs=7) as sb1p,  # bf16 [P,F] scratch
            tc.tile_pool(name="sb2", bufs=2) as sb2p,  # bf16 [P,F,2] scratch
            tc.tile_pool(name="sb3", bufs=2) as sb3p,  # bf16 [P,F,3] scratch
            tc.tile_pool(name="sq", bufs=2) as sqp,   # P3 squares
            tc.tile_pool(name="acc", bufs=1) as accp,
        ):
            def t_f32(name, k=None, short=True):
                if short:
                    return s1fp.tile([P, F], F32, tag="s1f",
                                     name=f"{name}{next(_tc_counter)}")
                shape = [P, F] if k is None else [P, F, k]
                return w1p.tile(shape, F32, tag=name, name=f"{name}_t")

            def t_bf(name, k=None, short=True):
                if short:
                    pool, tag = {None: (sb1p, "sb1"), 2: (sb2p, "sb2"),
                                 3: (sb3p, "sb3")}[k]
                    shape = [P, F] if k is None else [P, F, k]
                    return pool.tile(shape, BF16, tag=tag,
                                     name=f"{name}{next(_tc_counter)}")
                shape = [P, F] if k is None else [P, F, k]
                return wbp.tile(shape, BF16, tag=name, name=f"{name}_t")

            def _body():
                outacc = accp.tile([P, ncols], F32)
                nc.vector.memset(outacc, 0.0)

                q_ts, qh_ts, cov_ts = [], [], []
                for c in range(nch):
                    lo = c * F
                    q_t = inq.tile([P, F, 4], F32, tag="q")
                    qh_t = inq.tile([P, F, 4], F32, tag="qh")
                    nc.sync.dma_start(out=q_t, in_=qv[:, lo : lo + F, :])
                    nc.sync.dma_start(out=qh_t, in_=qhv[:, lo : lo + F, :])
                    cov_t = inp.tile([P, F, 16], F32, tag="cov")
                    nc.sync.dma_start(out=cov_t, in_=covv[:, lo : lo + F, :])
                    q_ts.append(q_t)
                    qh_ts.append(qh_t)
                    cov_ts.append(cov_t)

                recips, atans, tails = [], [], []

                for c in range(nch):
                    q_t, qh_t, cov_t = q_ts[c], qh_ts[c], cov_ts[c]
                    # ---------------- P1: th = wrap(q2 - atan2(qh2, qh3))
                    rxs = t_f32("rxs")
                    recips.append(_act_recip(nc, rxs[:, :], qh_t[:, :, 3]))
                    sg = t_f32("sg")
                    recips.append(nc.scalar.activation(
                        out=sg, in_=qh_t[:, :, 3], func=AT.Sign))
                    r = t_f32("r")
                    nc.vector.tensor_tensor(
                        out=r, in0=qh_t[:, :, 2], in1=rxs, op=OP.mult)
                    at = t_f32("at")
                    atans.append(nc.scalar.activation(
                        out=at, in_=r, func=AT.Arctan))
                    f1 = t_f32("f1")
                    nc.vector.scalar_tensor_tensor(
                        out=f1, in0=sg, scalar=PI / 2, in1=q_t[:, :, 2],
                        op0=OP.mult, op1=OP.add)
                    dp = t_f32("dp")  # delta + pi/2
                    nc.vector.tensor_tensor(
                        out=dp, in0=f1, in1=at, op=OP.subtract)
                    zi = t_f32("zi")
                    nc.vector.tensor_scalar(
                        out=zi.bitcast(I32), in0=dp, scalar1=1.0 / (2 * PI),
                        scalar2=-0.25, op0=OP.mult, op1=OP.add)
                    ff = t_f32("ff")
                    nc.vector.tensor_copy(ff, zi.bitcast(I32))
                    TH = t_f32("TH", short=False)  # th + pi/2
                    nc.vector.scalar_tensor_tensor(
                        out=TH, in0=ff, scalar=-2 * PI, in1=dp,
                        op0=OP.mult, op1=OP.add)
                    xy = t_bf("xy", 2, short=False)  # [x, y]
                    nc.vector.tensor_tensor(
                        out=xy, in0=q_t[:, :, 0:2], in1=qh_t[:, :, 0:2],
                        op=OP.subtract)

                    # ---------------- P2: Schur front (b, dt) + LDL
                    b3 = t_bf("b3", 3, short=False)
                    nc.gpsimd.tensor_tensor(
                        out=b3, in0=_sub_ap(cov_t, F, 3, 4, 3),
                        in1=_sub_ap(cov_t, F, 2, 4, 3), op=OP.subtract)
                    e1 = t_bf("e1")
                    nc.vector.tensor_tensor(
                        out=e1, in0=cov_t[:, :, 15], in1=cov_t[:, :, 11],
                        op=OP.subtract)
                    dtb = t_bf("dtb")
                    nc.vector.tensor_tensor(
                        out=dtb, in0=e1, in1=b3[:, :, 2], op=OP.subtract)
                    rdt = t_bf("rdt")
                    recips.append(_act_recip(nc, rdt[:, :], dtb[:, :]))
                    g3 = t_bf("g3", 3, short=False)
                    nc.vector.tensor_tensor(
                        out=g3, in0=b3[:, :, :], in1=_bc(rdt[:, :], 3),
                        op=OP.mult)
                    pr0 = t_bf("pr0", 3)
                    nc.vector.tensor_tensor(
                        out=pr0, in0=_bc(g3[:, :, 0], 3), in1=b3[:, :, :],
                        op=OP.mult)
                    s0 = t_bf("s0", 3, short=False)
                    eng_s = nc.gpsimd if O["pool_s01"] else nc.vector
                    eng_s.tensor_tensor(
                        out=s0, in0=cov_t[:, :, 0:3], in1=pr0, op=OP.subtract)
                    pr1 = t_bf("pr1", 2)
                    nc.vector.tensor_tensor(
                        out=pr1, in0=_bc(g3[:, :, 1], 2), in1=b3[:, :, 1:3],
                        op=OP.mult)
                    s1 = t_bf("s1", 2, short=False)
                    eng_s.tensor_tensor(
                        out=s1, in0=cov_t[:, :, 5:7], in1=pr1, op=OP.subtract)
                    pr2 = t_bf("pr2")
                    nc.vector.tensor_tensor(
                        out=pr2, in0=g3[:, :, 2], in1=b3[:, :, 2], op=OP.mult)
                    s22 = t_bf("s22")
                    nc.vector.tensor_tensor(
                        out=s22, in0=cov_t[:, :, 10], in1=pr2, op=OP.subtract)

                    r1 = t_bf("r1")
                    recips.append(_act_recip(nc, r1[:, :], s0[:, :, 0]))
                    L21 = t_bf("L21")
                    nc.vector.tensor_tensor(
                        out=L21, in0=s0[:, :, 1], in1=r1, op=OP.mult)
                    L31 = t_bf("L31")
                    nc.vector.tensor_tensor(
                        out=L31, in0=s0[:, :, 2], in1=r1, op=OP.mult)
                    pD0 = t_bf("pD0")
                    nc.vector.tensor_tensor(
                        out=pD0, in0=L21, in1=s0[:, :, 1], op=OP.mult)
                    pD1 = t_bf("pD1")
                    nc.vector.tensor_tensor(
                        out=pD1, in0=L21, in1=s0[:, :, 2], op=OP.mult)
                    D2 = t_bf("D2")
                    nc.vector.tensor_tensor(
                        out=D2, in0=s1[:, :, 0], in1=pD0, op=OP.subtract)
                    m32 = t_bf("m32")
                    nc.vector.tensor_tensor(
                        out=m32, in0=s1[:, :, 1], in1=pD1, op=OP.subtract)
                    r2 = t_bf("r2")
                    recips.append(_act_recip(nc, r2[:, :], D2[:, :]))
                    l32 = t_bf("l32")
                    nc.vector.tensor_tensor(
                        out=l32, in0=m32, in1=r2, op=OP.mult)
                    qa = t_bf("qa")
                    nc.vector.tensor_tensor(
                        out=qa, in0=s0[:, :, 2], in1=L31, op=OP.mult)
                    qb = t_bf("qb")
                    nc.vector.tensor_tensor(
                        out=qb, in0=m32, in1=l32, op=OP.mult)
                    d3a = t_bf("d3a")
                    nc.vector.tensor_tensor(
                        out=d3a, in0=s22, in1=qa, op=OP.subtract)
                    d3f = t_bf("d3f")
                    nc.vector.tensor_tensor(
                        out=d3f, in0=d3a, in1=qb, op=OP.subtract)
                    r3 = t_bf("r3")
                    recips.append(_act_recip(nc, r3[:, :], d3f[:, :]))

                    # ---------------- forward solve + quadratic form
                    pw0 = t_bf("pw0")
                    nc.vector.tensor_tensor(
                        out=pw0, in0=L21, in1=xy[:, :, 0], op=OP.mult)
                    pw1 = t_bf("pw1")
                    nc.vector.tensor_tensor(
                        out=pw1, in0=L31, in1=xy[:, :, 0], op=OP.mult)
                    w2 = t_bf("w2")
                    nc.vector.tensor_tensor(
                        out=w2, in0=xy[:, :, 1], in1=pw0, op=OP.subtract)
                    pw32 = t_bf("pw32")
                    nc.vector.tensor_tensor(
                        out=pw32, in0=l32, in1=w2, op=OP.mult)
                    ps3 = t_bf("ps3")
                    nc.vector.tensor_tensor(
                        out=ps3, in0=pw1, in1=pw32, op=OP.add)
                    w3 = t_bf("w3")
                    nc.vector.scalar_tensor_tensor(
                        out=w3, in0=TH, scalar=PI / 2, in1=ps3,
                        op0=OP.subtract, op1=OP.subtract)
                    w1sq = t_bf("w1sq")
                    if O["w1sq_pool"]:
                        nc.gpsimd.tensor_tensor(
                            out=w1sq, in0=xy[:, :, 0], in1=xy[:, :, 0],
                            op=OP.mult)
                    else:
                        nc.vector.tensor_tensor(
                            out=w1sq, in0=xy[:, :, 0], in1=xy[:, :, 0],
                            op=OP.mult)
                    w2sq = t_bf("w2sq")
                    nc.vector.tensor_tensor(
                        out=w2sq, in0=w2, in1=w2, op=OP.mult)
                    w3sq = t_bf("w3sq")
                    nc.vector.tensor_tensor(
                        out=w3sq, in0=w3, in1=w3, op=OP.mult)
                    for i, (wsq_t, r_t) in enumerate(
                            [(w1sq, r1), (w2sq, r2), (w3sq, r3)]):
                        po = t_bf(f"po{i}")
                        nc.vector.scalar_tensor_tensor(
                            out=po, in0=wsq_t, scalar=1.0, in1=r_t,
                            op0=OP.mult, op1=OP.mult,
                            accum_out=outacc[:, 3 * c + i : 3 * c + i + 1])

                    # ---------------- P3: l_cov = ln(sum C^2)
                    sq10 = sqp.tile([P, F, 10], BF16, tag="sq10")
                    if O["sq_diag_act"]:
                        tails.append(nc.scalar.activation(
                            out=sq10[:, :, 0:4], in_=_sub_ap(cov_t, F, 0, 5, 4),
                            func=AT.Square))
                    else:
                        nc.vector.tensor_tensor(
                            out=sq10[:, :, 0:4], in0=_sub_ap(cov_t, F, 0, 5, 4),
                            in1=_sub_ap(cov_t, F, 0, 5, 4), op=OP.mult)
                    nc.vector.scalar_tensor_tensor(
                        out=sq10[:, :, 4:7], in0=cov_t[:, :, 1:4], scalar=2.0,
                        in1=cov_t[:, :, 1:4], op0=OP.mult, op1=OP.mult)
                    nc.vector.scalar_tensor_tensor(
                        out=sq10[:, :, 7:9], in0=cov_t[:, :, 6:8], scalar=2.0,
                        in1=cov_t[:, :, 6:8], op0=OP.mult, op1=OP.mult)
                    nc.vector.scalar_tensor_tensor(
                        out=sq10[:, :, 9:10], in0=cov_t[:, :, 11:12], scalar=2.0,
                        in1=cov_t[:, :, 11:12], op0=OP.mult, op1=OP.mult)
                    t5 = sqp.tile([P, F, 5], BF16, tag="t5")
                    nc.vector.tensor_tensor(
                        out=t5, in0=sq10[:, :, 0:5], in1=sq10[:, :, 5:10],
                        op=OP.add)
                    t2 = t_bf("t2", 2)
                    eng_t2 = nc.gpsimd if O["tree_pool_t2"] else nc.vector
                    eng_t2.tensor_tensor(
                        out=t2, in0=t5[:, :, 0:2], in1=t5[:, :, 2:4], op=OP.add)
                    sa = t_bf("sa", short=False)
                    nc.vector.tensor_tensor(
                        out=sa, in0=t2[:, :, 0], in1=t2[:, :, 1], op=OP.add)
                    ssq = t_bf("ssq", short=False)
                    nc.vector.tensor_tensor(
                        out=ssq, in0=sa, in1=t5[:, :, 4], op=OP.add)
                    lnv = t_bf("lnv", short=False)
                    tails.append(nc.scalar.activation(
                        out=lnv, in_=ssq, func=AT.Ln,
                        accum_out=outacc[:, 3 * nch + c : 3 * nch + c + 1]))

                # ACT program-order hints: recips (reciprocal set) before
                # arctan of the NEXT chunk would force an extra table swap;
                # natural emission order above is already grouped per chunk.
                # Force squares/lns after all chunk recips to save swaps.
                if O["act_hints"]:
                    for t_i in tails:
                        for r_i in recips:
                            tile.add_dep_helper(t_i.ins, r_i.ins, sync=False,
                                                reason="act set order")
                nc.sync.dma_start(out=out[:, :], in_=outacc)

            if repeat > 1:
                with tc.For_i(0, repeat, 1):
                    _body()
            else:
                _body()

    if split_waits:
        split_multi_waits(nc)
    return nc, ncols, nch


VARIANT = 2

_CACHE = {}


def _get_nc():
    if "nc" not in _CACHE:
        if VARIANT == 2:
            nc, ncols, nch = build_nc_v2()
            _CACHE["nc"] = (nc, ncols, 3 * nch, nch)
        else:
            _CACHE["nc"] = build_nc()
    return _CACHE["nc"]


def kernel(q, q_hat, cov, device=0, _return_raw=False):
    nc, ncols, nch, nch3 = _get_nc()
    N = int(np.prod(q.shape[:-1]))
    rows = N // NCORES  # elements per core
    qf = np.ascontiguousarray(np.asarray(q).reshape(N, 4), dtype=np.float32)
    qhf = np.ascontiguousarray(np.asarray(q_hat).reshape(N, 4), dtype=np.float32)
    covf = np.ascontiguousarray(np.asarray(cov).reshape(N, 16), dtype=np.float32)
    in_maps = []
    for k in range(NCORES):
        sl = slice(k * rows, (k + 1) * rows)
        in_maps.append(
            {
                "q": qf[sl].reshape(P, -1),
                "q_hat": qhf[sl].reshape(P, -1),
                "cov": covf[sl].reshape(P, -1),
            }
        )
    res = run_bass_kernel_spmd(nc, in_maps, list(range(NCORES)))
    outs = np.stack([np.asarray(res.results[k]["out"]) for k in range(NCORES)])
    if _return_raw:
        return outs
    S = outs.astype(np.float64)
    qf_sum = S[:, :, 0:nch].sum()
    ln_sum = S[:, :, nch : nch + nch3].sum()
    total = (0.5 * qf_sum + 0.25 * ln_sum) / float(N)
    return np.array(total, dtype=np.float32)

